# revision 5
# baseline (speedup 1.0000x reference)
"""Hyperbolic (Poincare-ball) average pooling 1D — Trainium2 Bass kernel, v2.

Problem: x (16, 256, 16384) f32, kernel=stride=4, manifold dim = channels (256).
Math (c=1), per window position:
    n2   = sum_C x^2                     (per input position)
    r    = 1/(1-n2)                      (gamma*xK = 2*r*x ; gamma = 2r-1)
    num  = sum_j r_j x_j  (window of 4)  ; den = sum_j r_j ; D = den - 2
    out  = num * g,  g = 1/(D + sqrt(D^2 - s)),  s = sum_C num^2

v2 structure (all engine costs measured on hw):
  - big tiles: [128, 16*256] bf16 (2048 positions), 16 tiles/core.
  - sq = x*x via ONE DVE TT with two structurally-different AP views of the
    same bytes (true 2x mode: 1.2us vs 1.55us for identical APs).
  - first reduction fold runs on a DMA queue: gpsimd software-DGE dma with
    accum_op=add does sq[:,0:2048] += sq[:,2048:4096] off-engine.
  - t2 fold + 65-col negated tensor_reduce (-1 carry col) -> 1-n2 -> recip.
  - window-sum on PE via banded matmul (baseline trick): each q-slot's 128
    positions fold into a disjoint 32-row band; band = mask * r broadcast
    (one Pool op per tile).  den = sum r via separate 1-col matmuls against
    a constant ones vector into a tiny [128,4] PSUM tile, so x tiles carry
    no ones column and the square sees a clean [128,4096] block.
  - s via ScalarE Square+accum off PSUM; D via ONE [128,4] Identity(bias=-2);
    g-chain tiny ops on DVE; out = PSUM * g via ScalarE Copy(scale) (3 of 4)
    and DVE tensor_scalar (1 of 4) to balance engines.
  - ~5-stage software pipeline: [load+sq+folddma](k) | [t2+reduce+recip](k-2)
    | post(k-3) | mm+sD(k-3); the fold dma gets two iterations of queue
    slack, warmup tiles fold on DVE so ramp is not dma-gated, and the last
    two tiles' output scaling runs on DVE (ScalarE is the serial drain).

Sharding: data-parallel over batch (2 rows/core, 8 cores). Host pre-transposes
each core's slice to (positions, channels) bf16.
"""

import sys

sys.path.insert(0, "/opt/trn_rl_repo")

import copy
import numpy as np
import ml_dtypes

import bass_rust
import concourse.bass as bass
import concourse.mybir as mybir
from concourse import tile
from concourse.bass_utils import run_bass_kernel_spmd
from contextlib import ExitStack

F32 = mybir.dt.float32
BF16 = mybir.dt.bfloat16
NP_BF16 = ml_dtypes.bfloat16

B, C, L = 16, 256, 16384
KERN = 4
T = L // KERN            # 4096 out positions per batch row
N_CORES = 8
B_PER = B // N_CORES     # 2
POS = B_PER * L          # 32768 input positions per core
OPOS = POS // KERN       # 8192 out positions per core
Q = 16                   # q-slots per x-tile
TILE_POS = 128 * Q       # 2048 input positions per x-tile
N_TILES = POS // TILE_POS  # 16
OUT_PT = 4 * 256         # out cols per tile in p-major dram layout

AF = mybir.ActivationFunctionType
ALU = mybir.AluOpType


def _split_multi_waits(nc, max_waits=1):
    """walrus in this container rejects >1 sync-wait on one instruction
    (setupSyncWait 'Too many sync wait commands'); split extras into
    preceding single-wait NOPs on the same engine."""
    n_new = 0
    for bb in nc.m.functions[0].blocks:
        new_list = []
        for inst in bb.instructions:
            si = getattr(inst, "sync_info", None)
            if si is not None and si.on_wait and len(si.on_wait) > max_waits:
                extra = si.on_wait[:-max_waits]
                si_keep = si.on_wait[-max_waits:]
                for w in extra:
                    nop = mybir.InstNoOp(
                        name=f"{inst.name}-wsplit{n_new}", ins=[], outs=[]
                    )
                    nop.engine = inst.engine
                    nsi = copy.deepcopy(si)
                    nsi.on_wait = [w]
                    nsi.on_update = []
                    nop.sync_info = nsi
                    new_list.append(nop)
                    n_new += 1
                si.on_wait = si_keep
            new_list.append(inst)
        bb.instructions = new_list
    return n_new


def _register_const_ap(nc, value):
    t = nc.alloc_sbuf_tensor(f"const-float32-{value}", [128, 1], F32)
    nc.gpsimd.memset(t.ap(), value)
    nc.const_aps.aps[(F32, value)] = t.ap()


def build_nc(split_waits=True):
    nc = bass.Bass()
    _register_const_ap(nc, -2.0)
    nc.all_engine_barrier()
    xt = nc.declare_dram_parameter("xt", [N_TILES, 128, Q * C], BF16, isOutput=False)
    mb = nc.declare_dram_parameter("mb", [128, 32], BF16, isOutput=False)
    # p-major output so paired tiles DMA with matching iteration order
    out = nc.declare_dram_parameter("out", [N_TILES, 128, OUT_PT], BF16, isOutput=True)

    with tile.TileContext(nc) as tc:
        with ExitStack() as ctx:
            xpool = ctx.enter_context(tc.tile_pool(name="x", bufs=7))
            sqpool = ctx.enter_context(tc.tile_pool(name="sq", bufs=4))
            t2pool = ctx.enter_context(tc.tile_pool(name="t2", bufs=3))
            bdpool = ctx.enter_context(tc.tile_pool(name="bd", bufs=3))
            stpool = ctx.enter_context(tc.tile_pool(name="st", bufs=3))
            scpool = ctx.enter_context(tc.tile_pool(name="sc", bufs=4))
            opool = ctx.enter_context(tc.tile_pool(name="o", bufs=3))
            mkpool = ctx.enter_context(tc.tile_pool(name="mk", bufs=1))
            pspool = ctx.enter_context(tc.tile_pool(name="ps", bufs=7, space="PSUM"))
            dnpool = ctx.enter_context(tc.tile_pool(name="dn", bufs=1, space="PSUM"))

            mb_t = mkpool.tile([128, 32], BF16, tag="mb")
            nc.sync.dma_start(mb_t[:], mb[:, :])
            mb_bc = (
                mb_t[:]
                .rearrange("p (a b t) -> p a b t", a=1, b=1)
                .broadcast_to([128, 8, 2, 32])
            )
            ones1 = mkpool.tile([128, 1], BF16, tag="ones")
            nc.vector.memset(ones1[:], 1.0)

            # [128, Q, 64] band tiles: slot q's 32-wide band sits at column
            # 32*(q%2); zeros elsewhere are written once and persist across
            # the pool's buffer rotation.
            def band_ap(wt_full):
                # band offset for q = 2a+b is 64q + 32b = 128a + 96b
                return bass_rust.AP(
                    tensor=wt_full.tensor,
                    offset=wt_full.offset,
                    ap=[list(wt_full.ap[0]), [128, 8], [96, 2], [1, 32]],
                )

            for _ in range(3):
                wt = bdpool.tile([128, Q, 64], BF16, tag="bd")
                nc.vector.memset(wt[:], 0.0)
            # t2 carries a constant -1 in col 64 so the negated reduce
            # yields 1-n2 directly
            for _ in range(3):
                t2i = t2pool.tile([128, Q, 65], BF16, tag="t2")
                nc.gpsimd.memset(t2i[:, :, 64:65], -1.0)

            def stage1a(i):
                """load, square (one 2x TT), dma-queue fold."""
                x_t = xpool.tile([128, Q * C], BF16, tag="x")
                nc.sync.dma_start(x_t[:], xt[i])
                sq_t = sqpool.tile([128, Q * C], BF16, tag="sq")
                # two structurally different views of the same bytes ->
                # hardware 2x mode (identical APs measure ~1.3x slower)
                nc.vector.tensor_tensor(
                    out=sq_t[:],
                    in0=x_t[:],
                    in1=x_t[:].rearrange("p (a c) -> p a c", a=2),
                    op=ALU.mult,
                )
                # first fold is split: the swdge CCE queue is slow (~10us
                # for a full-tile fold), so dma folds slots 0..7 and the DVE
                # folds slots 8..15 in stage1b.  First two tiles fold fully
                # on DVE so pipeline ramp is not gated on the dma.
                if i >= 2:
                    nc.gpsimd.dma_start(
                        sq_t[:, 0:1536],
                        sq_t[:, 2048:3584],
                        accum_op=ALU.add,
                    )
                else:
                    nc.vector.tensor_tensor(
                        out=sq_t[:, 0:1536],
                        in0=sq_t[:, 0:1536],
                        in1=sq_t[:, 2048:3584],
                        op=ALU.add,
                    )
                return x_t, sq_t

            def stage1b(sq_t):
                """t2 fold + negated reduce -> 1-n2 -> r (one iter after
                stage1a so the fold dma has a full stage of slack)."""
                nc.vector.tensor_tensor(
                    out=sq_t[:, 1536:2048],
                    in0=sq_t[:, 1536:2048],
                    in1=sq_t[:, 3584:4096],
                    op=ALU.add,
                )
                fold = sq_t[:, 0 : Q * C // 2].rearrange("p (q c) -> p q c", q=Q)
                t2_t = t2pool.tile([128, Q, 65], BF16, tag="t2")
                nc.vector.tensor_tensor(
                    out=t2_t[:, :, 0:64],
                    in0=fold[:, :, 0:64],
                    in1=fold[:, :, 64:128],
                    op=ALU.add,
                )
                n2_g = stpool.tile([128, Q], F32, tag="n2")
                nc.vector.tensor_reduce(
                    n2_g[:],
                    t2_t[:],
                    axis=mybir.AxisListType.X,
                    op=ALU.add,
                    negate=True,
                )
                r_g = stpool.tile([128, Q], F32, tag="r")
                nc.vector.reciprocal(r_g[:], n2_g[:])
                return r_g

            def emit_mm(x_t, r_g):
                """band build + 16 banded matmuls + 16 den matmuls."""
                band = bdpool.tile([128, Q, 64], BF16, tag="bd")
                r_bc = (
                    r_g[:]
                    .rearrange("p (a b o) -> p a b o", a=8, o=1)
                    .broadcast_to([128, 8, 2, 32])
                )
                nc.gpsimd.tensor_tensor(
                    out=band_ap(band[:].rearrange("p q t -> p (q t)")),
                    in0=mb_bc,
                    in1=r_bc,
                    op=ALU.mult,
                )
                # channels of slot q live at cols {h*2048 + q*128 + c128}
                xq = x_t[:].rearrange("p (h q c) -> p q h c", h=2, q=Q)
                ps_list = []
                dn = dnpool.tile([128, 4], F32, tag="dn")
                for bk in range(4):
                    for ql in range(4):
                        q = 4 * bk + ql
                        nc.tensor.matmul(
                            dn[64 * (ql // 2) : 64 * (ql // 2) + 64, bk : bk + 1],
                            band[:, q, :],
                            ones1[:],
                            start=(ql % 2 == 0),
                            stop=(ql % 2 == 1),
                        )
                for bk in range(4):
                    ps = pspool.tile([128, 256], F32, tag="ps")
                    for ql in range(4):
                        q = 4 * bk + ql
                        nc.tensor.matmul(
                            ps[64 * (ql // 2) : 64 * (ql // 2) + 64, :],
                            band[:, q, :],
                            xq[:, q, :],
                            start=(ql % 2 == 0),
                            stop=(ql % 2 == 1),
                        )
                    ps_list.append(ps)
                return ps_list, dn

            def emit_sD(ps_list, dn):
                """s = sum num^2 per psum tile; D = den-2 (one [128,4] op)."""
                d_s = scpool.tile([128, 4], F32, tag="d")
                s_s = scpool.tile([128, 4], F32, tag="s")
                for col, ps in enumerate(ps_list):
                    sq_scr = scpool.tile([128, 256], BF16, tag="sqs")
                    nc.scalar.activation(
                        sq_scr[:],
                        ps[:],
                        AF.Square,
                        accum_out=s_s[:, col : col + 1],
                    )
                nc.scalar.activation(d_s[:], dn[:], AF.Identity, bias=-2.0)
                return d_s, s_s

            def emit_post(i, ps_list, d_s, s_s):
                """g-chain (DVE tiny ops + Act sqrt), output scale + DMA."""
                d2 = scpool.tile([128, 4], F32, tag="d2")
                nc.scalar.activation(d2[:], d_s[:], AF.Square)
                qq = scpool.tile([128, 4], F32, tag="qq")
                nc.gpsimd.tensor_tensor(out=qq[:], in0=d2[:], in1=s_s[:], op=ALU.subtract)
                u = scpool.tile([128, 4], F32, tag="u")
                nc.scalar.activation(u[:], qq[:], AF.Sqrt)
                du = scpool.tile([128, 4], F32, tag="du")
                nc.vector.tensor_tensor(out=du[:], in0=d_s[:], in1=u[:], op=ALU.add)
                g_s = scpool.tile([128, 4], F32, tag="g")
                nc.vector.reciprocal(g_s[:], du[:])

                o_t = opool.tile([128, 4, 256], BF16, tag="o")
                for col in range(4):
                    if i >= N_TILES - 2:
                        nc.vector.tensor_scalar(
                            out=o_t[:, col, :],
                            in0=ps_list[col][:],
                            scalar1=g_s[:, col : col + 1],
                            scalar2=None,
                            op0=ALU.mult,
                        )
                    else:
                        nc.scalar.activation(
                            o_t[:, col, :],
                            ps_list[col][:],
                            AF.Copy,
                            scale=g_s[:, col : col + 1],
                        )
                nc.sync.dma_start(out[i], o_t[:].rearrange("p a c -> p (a c)"))

            prev = None      # (i, x_t, r_g) awaiting matmuls
            pending = None   # (i, ps_list, d_s, s_s) awaiting g-chain/out
            laq = []      # [(i, x_t, sq_t)] awaiting t2/reduce/recip (2 slots)
            lb = None     # (i, x_t, r_g) awaiting matmuls
            pending = None  # (i, ps_list, d_s, s_s) awaiting g-chain/out

            def flush_b(lb_):
                nonlocal pending
                if pending is not None:
                    emit_post(*pending)
                    pending = None
                if lb_ is not None:
                    bi, x_t_b, r_b = lb_
                    ps_list, dn = emit_mm(x_t_b, r_b)
                    d_s, s_s = emit_sD(ps_list, dn)
                    pending = (bi, ps_list, d_s, s_s)

            for i in range(N_TILES):
                a = stage1a(i)
                laq.append((i, *a))
                nlb = None
                slack = 1 if i <= 2 else 2
                if len(laq) > slack:
                    ai, x_t_a, sq_a = laq.pop(0)
                    r_g = stage1b(sq_a)
                    nlb = (ai, x_t_a, r_g)
                if i <= 2:
                    # warmup: run mm in the same iteration as its stage1b
                    flush_b(nlb)
                    lb = None
                else:
                    flush_b(lb)
                    lb = nlb
            # drain
            while laq:
                ai, x_t_a, sq_a = laq.pop(0)
                r_g = stage1b(sq_a)
                flush_b(lb)
                lb = (ai, x_t_a, r_g)
            flush_b(lb)
            if pending is not None:
                emit_post(*pending)

    if split_waits:
        _split_multi_waits(nc)
    return nc


_NC_CACHE = None


def _get_nc():
    global _NC_CACHE
    if _NC_CACHE is None:
        _NC_CACHE = build_nc()
    return _NC_CACHE


def _make_mask():
    m = np.zeros((128, 32), dtype=NP_BF16)
    m[np.arange(128), np.arange(128) // 4] = 1.0
    return m


def prepare_core_inputs(x):
    """x: (16, 256, 16384) f32 -> list of per-core input dicts."""
    mask = _make_mask()
    in_maps = []
    for k in range(N_CORES):
        xs = x[k * B_PER : (k + 1) * B_PER]  # (2, 256, L)
        xtp = xs.transpose(0, 2, 1).reshape(POS, C).astype(NP_BF16)
        # partition-major per-tile layout: (tile, p, q*C)
        # col = h*2048 + q*128 + c128 (channel = h*128 + c128) so the
        # dma-fold's two 2048-halves pair channel halves of the SAME slot
        xtp = np.ascontiguousarray(
            xtp.reshape(N_TILES, Q, 128, 2, 128).transpose(0, 2, 3, 1, 4)
        ).reshape(N_TILES, 128, Q * C)
        in_maps.append({"xt": xtp, "mb": mask})
    return in_maps


def assemble_output(results):
    outs = []
    for k in range(N_CORES):
        o = results[k]["out"]  # (N_TILES, 128, 4*256) bf16
        o = np.asarray(o).astype(np.float32)
        # o[i, m, h*256+c] -> out position i*512 + h*128 + m, channel c
        o = o.reshape(N_TILES, 128, 4, 256).transpose(0, 2, 1, 3).reshape(OPOS, 256)
        outs.append(o.reshape(B_PER, T, C).transpose(0, 2, 1))
    return np.ascontiguousarray(np.concatenate(outs, axis=0))


def kernel(x):
    x = np.ascontiguousarray(x, dtype=np.float32)
    nc = _get_nc()
    in_maps = prepare_core_inputs(x)
    res = run_bass_kernel_spmd(nc, in_maps, core_ids=list(range(N_CORES)))
    return assemble_output(res.results)
